# revision 1
# baseline (speedup 1.0000x reference)
"""LocalScoreMachine Trainium2 kernel.

score[b,c,p] = -sum_n w[b,n,p]*(x[b,c,p]-m*I[n,c,p]) / (sig2 * sum_n w[b,n,p])
with w = exp(-box3(|x - m*I|^2 summed over c)/(2*sig2) - sub).

Expansion: box3(norm) = box3(A) + m^2*box3(S) - 2m*box3(z),
A = sum_c x_c^2 (b-only), S = sum_c I_c^2 (n-only), z = sum_c x_c*I_c.
The exp factor from box3(A) (and any per-(b,p) stabilizer) is constant in n,c
and cancels in the numerator/denominator ratio, so each core computes
    w' = exp(box3((m/sig2)*z - (m^2/(2 sig2))*S))
over its shard of N, accumulates SW = sum_n w', SWI_c = sum_n w'*I_c via
TensorE ones-matmuls, and the host combines 8 partial results:
    score = (m*SWI/SW - x)/sig2.

Sharding: dataset axis N=2048 -> 256 images per core (8 cores), as 2 tiles of
[128 partitions = n, (3,32,32) free].
"""

import sys

for _p in ("/opt/trn_rl_repo", "/opt/trn_rl_repo/concourse", "/opt/pypackages"):
    if _p not in sys.path:
        sys.path.append(_p)

from contextlib import ExitStack

import numpy as np

import concourse.bass as bass
import concourse.bacc as bacc
import concourse.mybir as mybir
import concourse.tile as tile
from concourse import bass_utils

B, N, C, H, W = 8, 2048, 3, 32, 32
P = H * W  # 1024 pixels
NCORES = 8
NLOC = N // NCORES  # 256
NT = NLOC // 128  # 2 partition tiles per core
F32 = mybir.dt.float32
AF = mybir.ActivationFunctionType

_cache = {}
_last_res = None


def _build(m: float, sig2: float):
    """Build + compile the per-core SPMD program. m, sig2 are compile-time."""
    nc = bacc.Bacc("TRN2", target_bir_lowering=False, debug=False)

    img_d = nc.dram_tensor("img", [NLOC, C, H, W], F32, kind="ExternalInput")
    xs_d = nc.dram_tensor("xs", [B, C * P], F32, kind="ExternalInput")
    out_d = nc.dram_tensor("out", [B, 4, P], F32, kind="ExternalOutput")

    c_s = -(m * m) / (2.0 * sig2)  # multiplies S
    # z scale m/sig2 is folded into xs on the host.

    with tile.TileContext(nc) as tc, ExitStack() as ctx:
        const = ctx.enter_context(tc.tile_pool(name="const", bufs=1))
        imgs = ctx.enter_context(tc.tile_pool(name="imgs", bufs=1))
        spool = ctx.enter_context(tc.tile_pool(name="spool", bufs=1))
        pre = ctx.enter_context(tc.tile_pool(name="pre", bufs=1))
        xrs_pool = ctx.enter_context(tc.tile_pool(name="xrs", bufs=4))
        workv = ctx.enter_context(tc.tile_pool(name="workv", bufs=2))
        work = ctx.enter_context(tc.tile_pool(name="work", bufs=3))
        psum = ctx.enter_context(
            tc.tile_pool(name="psum", bufs=2, space=bass.MemorySpace.PSUM)
        )
        acc_pool = ctx.enter_context(
            tc.tile_pool(name="acc", bufs=1, space=bass.MemorySpace.PSUM)
        )

        ones_row = const.tile([1, 128], F32)  # lhsT for broadcast (K=1,M=128)
        ones_col = const.tile([128, 32], F32)  # lhsT for reduction (K=128,M=32)
        nc.gpsimd.memset(ones_row[:], 1.0)
        nc.gpsimd.memset(ones_col[:], 1.0)



        img_ap = img_d.ap().rearrange("(t p) c h w -> t p (c h w)", p=128)
        itiles = []
        spp = []
        for t in range(NT):
            it = imgs.tile([128, C, P], F32, tag=f"img{t}", name=f"img{t}")
            nc.sync.dma_start(it[:], img_ap[t])
            itiles.append(it)

            # S'' = c_s * sum_c I_c^2
            sq = pre.tile([128, C, P], F32, tag="sq")
            nc.scalar.square(sq[:], it[:])
            s0 = work.tile([128, P], F32, tag="tmp")
            nc.vector.tensor_add(s0[:], sq[:, 0], sq[:, 1])
            s1 = work.tile([128, P], F32, tag="chain")
            nc.vector.tensor_add(s1[:], s0[:], sq[:, 2])
            sp = spool.tile([128, P], F32, tag=f"spp{t}", name=f"spp{t}")
            nc.vector.tensor_scalar_mul(sp[:], s1[:], c_s)
            spp.append(sp)

        for b in range(B):
            # stage xs[b] on partition 0, then broadcast via PE ones-matmul
            xsb = workv.tile([1, C * P], F32, tag="xsb", name=f"xsb_{b}")
            nc.sync.dma_start(xsb[:], xs_d.ap()[b][None, :])
            xrc = []
            for c in range(C):
                xp = psum.tile([128, P], F32, tag="xr", name=f"xr_{b}_{c}")
                for half in range(2):
                    nc.tensor.matmul(
                        xp[:, half * 512 : (half + 1) * 512],
                        ones_row[:],
                        xsb[0:1, c * P + half * 512 : c * P + half * 512 + 512],
                    )
                xr_sb = xrs_pool.tile([128, P], F32, tag="xrs", name=f"xrs_{b}_{c}")
                nc.scalar.copy(xr_sb[:], xp[:])
                xrc.append(xr_sb)

            # accumulators: quadrant-packed redundant-row [32,512] blocks
            # accq[half] rows: 0-31=SW, 32-63=SWI0, 64-95=SWI1; accr[half]=SWI2
            accq0 = acc_pool.tile([96, 512], F32, tag="accq0")
            accq1 = acc_pool.tile([96, 512], F32, tag="accq1")
            accr0 = acc_pool.tile([32, 512], F32, tag="accr0")
            accr1 = acc_pool.tile([32, 512], F32, tag="accr1")
            accq = [accq0, accq1]
            accr = [accr0, accr1]

            for t in range(NT):
                it = itiles[t]
                # u = S'' + sum_c I_c * xs_c   (xs pre-scaled by m/sig2)
                t0 = work.tile([128, P], F32, tag="tmp")
                nc.vector.tensor_mul(t0[:], it[:, 0], xrc[0][:])
                u0 = work.tile([128, P], F32, tag="chain")
                nc.vector.tensor_add(u0[:], t0[:], spp[t][:])
                t1 = work.tile([128, P], F32, tag="tmp")
                nc.vector.tensor_mul(t1[:], it[:, 1], xrc[1][:])
                u1 = work.tile([128, P], F32, tag="chain")
                nc.vector.tensor_add(u1[:], u0[:], t1[:])
                t2 = work.tile([128, P], F32, tag="tmp")
                nc.vector.tensor_mul(t2[:], it[:, 2], xrc[2][:])
                u = work.tile([128, H, W], F32, tag="chain")
                nc.vector.tensor_add(
                    u[:].rearrange("p h w -> p (h w)"), u1[:], t2[:]
                )

                # separable 3x3 box filter (zero pad), free dims (h, w)
                r = work.tile([128, H, W], F32, tag="tmp")  # t[w] = u[w]+u[w+1]
                nc.vector.tensor_add(r[:, :, 0:31], u[:, :, 0:31], u[:, :, 1:32])
                nc.scalar.copy(r[:, :, 31:32], u[:, :, 31:32])
                r2 = work.tile([128, H, W], F32, tag="chain")  # rowsum
                nc.vector.tensor_add(r2[:, :, 1:32], r[:, :, 1:32], u[:, :, 0:31])
                nc.scalar.copy(r2[:, :, 0:1], r[:, :, 0:1])

                s = work.tile([128, H, W], F32, tag="tmp")  # t2[h] = r2[h]+r2[h+1]
                nc.vector.tensor_add(s[:, 0:31, :], r2[:, 0:31, :], r2[:, 1:32, :])
                nc.scalar.copy(s[:, 31:32, :], r2[:, 31:32, :])
                arg = work.tile([128, H, W], F32, tag="chain")  # full box sum
                nc.vector.tensor_add(arg[:, 1:32, :], s[:, 1:32, :], r2[:, 0:31, :])
                nc.scalar.copy(arg[:, 0:1, :], s[:, 0:1, :])

                wt = work.tile([128, H, W], F32, tag="wt")
                nc.scalar.activation(wt[:], arg[:], AF.Exp)

                v = workv.tile([128, C, P], F32, tag="v")
                wflat = wt[:].rearrange("p h w -> p (h w)")
                for c in range(C):
                    nc.vector.tensor_mul(v[:, c], wflat, it[:, c])

                # reduce over n (partitions) via ones matmuls, accumulate in PSUM
                first, last = (t == 0), (t == NT - 1)
                for half in range(2):
                    sl = slice(half * 512, (half + 1) * 512)
                    nc.tensor.matmul(
                        accq[half][0:32], ones_col[:], wflat[:, sl],
                        start=first, stop=last,
                    )
                    nc.tensor.matmul(
                        accq[half][32:64], ones_col[:], v[:, 0, sl],
                        start=first, stop=last,
                    )
                    nc.tensor.matmul(
                        accq[half][64:96], ones_col[:], v[:, 1, sl],
                        start=first, stop=last,
                    )
                    nc.tensor.matmul(
                        accr[half][0:32], ones_col[:], v[:, 2, sl],
                        start=first, stop=last,
                    )

            for half in range(2):
                sl = slice(half * 512, (half + 1) * 512)
                oq = work.tile([96, 512], F32, tag="oq", name=f"oq_{b}_{half}")
                nc.scalar.copy(oq[:], accq[half][:])
                orr = work.tile([32, 512], F32, tag="orr", name=f"orr_{b}_{half}")
                nc.scalar.copy(orr[:], accr[half][:])
                nc.sync.dma_start(out_d.ap()[b, 0, sl], oq[0:1, :])
                nc.sync.dma_start(out_d.ap()[b, 1, sl], oq[32:33, :])
                nc.sync.dma_start(out_d.ap()[b, 2, sl], oq[64:65, :])
                nc.sync.dma_start(out_d.ap()[b, 3, sl], orr[0:1, :])

    nc.compile()
    return nc


def kernel(x, images, mu, sigma, t):
    x = np.ascontiguousarray(np.asarray(x, dtype=np.float32))
    images = np.ascontiguousarray(np.asarray(images, dtype=np.float32))
    m = float(np.asarray(mu)[int(t)])
    sig = float(np.asarray(sigma)[int(t)])
    sig2 = sig * sig

    key = (m, sig2)
    if key not in _cache:
        _cache[key] = _build(m, sig2)
    nc = _cache[key]

    xs = (x.reshape(B, C * P) * (m / sig2)).astype(np.float32)
    imgs = images.reshape(N, C * P)
    in_maps = []
    for k in range(NCORES):
        in_maps.append(
            {
                "img": np.ascontiguousarray(
                    imgs[k * NLOC : (k + 1) * NLOC].reshape(NLOC, C, H, W)
                ),
                "xs": xs,
            }
        )

    import os
    trace = bool(os.environ.get("KERNEL_TRACE"))
    res = bass_utils.run_bass_kernel_spmd(
        nc, in_maps, core_ids=list(range(NCORES)), trace=trace
    )
    global _last_res
    _last_res = res
    parts = np.stack([res.results[k]["out"] for k in range(NCORES)])  # [8,B,4,P]
    tot = parts.sum(axis=0)
    sw = tot[:, 0, :]  # [B,P]
    swi = tot[:, 1:4, :]  # [B,C,P]
    score = (m * swi / sw[:, None, :] - x.reshape(B, C, P)) / sig2
    return score.reshape(B, C, H, W).astype(np.float32)



# revision 9
# speedup vs baseline: 2.6585x; 2.6585x over previous
"""LocalScoreMachine Trainium2 kernel (pixel-major frontend + PE box filter).

score[b,c,p] = (m*SWI_c/SW - x[b,c,p]) / sig2, where over the dataset axis n:
    SW  = sum_n w,  SWI_c = sum_n w*I_c,
    w   = exp(box3(u)),  u = sum_c I_c * x'_c + c_s*S,
    x'  = x*(m/sig2),  S = sum_c I_c^2,  c_s = -m^2/(2*sig2).
(The b- and n-constant exp factors cancel in the SWI/SW ratio, and the
3x3 box filter is linear, so box3(u) gives the exact exponent up to that
constant.)

Sharding: dataset axis N=2048 -> 256 images per core (8 cores).

Per-core dataflow:
  *P-major phase* (partitions = 128 pixels of a 4-row chunk, free = n):
    - t_c = I_c * x'_c via tensor_scalar muls (per-partition scalar = x'),
      which hit the DVE 4x perf mode in bf16.  Split across DVE/Act/Pool.
    - u = t0+t1+t2+S'' via wide tensor adds (free = all 8 chunks * 256 n).
    - box3 via PE matmuls: block-tridiagonal 0/1 band matrices B_delta
      [128x128] couple chunk ci with chunks ci+delta; zero guard slots
      implement the zero padding.  Accumulated in PSUM.
    - w = exp(arg) on the Act engine, PSUM -> SBUF bf16.
  *Transpose*: one SBUF->SBUF dma_start_transpose per b flips w to
    image-major layout [n partitions, pixel free].
  *A-major phase* (partitions = n): v_c = w*I_c muls, then ones-matmul
    reductions over n on the PE into PSUM, accumulated over both n-tiles.
Host combines the 8 partial (SW, SWI) results and forms the score.
"""

import sys

for _p in ("/opt/trn_rl_repo", "/opt/trn_rl_repo/concourse", "/opt/pypackages"):
    if _p not in sys.path:
        sys.path.append(_p)

from contextlib import ExitStack

import numpy as np
import ml_dtypes

import concourse.bass as bass
import concourse.bacc as bacc
import concourse.mybir as mybir
import concourse.tile as tile
from concourse import bass_utils

B, N, C, H, W = 8, 2048, 3, 32, 32
P = H * W  # 1024 pixels
NCORES = 8
NLOC = N // NCORES  # 256 images per core
NT = 2  # n-tiles (128 partitions) per core on the A-major side
CI = 8  # pixel chunks (4 h-rows x 32 w = 128 pixels each)
Q = 128  # pixels per chunk
F32 = mybir.dt.float32
BF16 = mybir.dt.bfloat16
AF = mybir.ActivationFunctionType
BF_NP = ml_dtypes.bfloat16

_cache = {}
_last_res = None

# Engine assignment for the 24 per-b tensor_scalar muls (c, ci), tuned so
# DVE/Act/Pool finish together (DVE ~127ns/op, Act ~398, Pool ~451).
# index = c * CI + ci -> "d" (DVE) / "a" (Act) / "p" (Pool)
_MUL_ENG = list("ddaddpdd" "dpadddpd" "ddpaddpd")


def _box_mats():
    """B_delta[k, m] = 1 if source pixel k (in chunk ci+delta) is in the
    3x3 neighborhood of target pixel m (in chunk ci)."""
    q = np.arange(Q)
    hk, wk = q // W, q % W  # chunk-local h (0..3), w (0..31)
    hm, wm = hk, wk
    mats = []
    for delta in (-1, 0, 1):
        dh = 4 * delta + hk[:, None] - hm[None, :]
        dw = wk[:, None] - wm[None, :]
        mats.append(((np.abs(dh) <= 1) & (np.abs(dw) <= 1)).astype(np.float32))
    return np.stack(mats)  # [3, 128, 128] (k, m)


def _build():
    nc = bacc.Bacc("TRN2", target_bir_lowering=False, debug=False)

    ip_d = nc.dram_tensor("ip", [Q, C, CI, NLOC], BF16, kind="ExternalInput")
    ia_d = nc.dram_tensor("ia", [NT, 128, C, CI, Q], BF16, kind="ExternalInput")
    sp_d = nc.dram_tensor("sp", [Q, CI, NLOC], BF16, kind="ExternalInput")
    xsc_d = nc.dram_tensor("xsc", [Q, B, C, CI], F32, kind="ExternalInput")
    bm_d = nc.dram_tensor("bm", [3, Q, Q], BF16, kind="ExternalInput")
    out_d = nc.dram_tensor("out", [B, 9, 512], F32, kind="ExternalOutput")

    with tile.TileContext(nc) as tc, ExitStack() as ctx:
        const = ctx.enter_context(tc.tile_pool(name="const", bufs=1))
        work = ctx.enter_context(tc.tile_pool(name="work", bufs=2))
        wpool = ctx.enter_context(tc.tile_pool(name="wpool", bufs=2))
        psum = ctx.enter_context(
            tc.tile_pool(name="psum", bufs=2, space=bass.MemorySpace.PSUM)
        )
        rpsum = ctx.enter_context(
            tc.tile_pool(name="rpsum", bufs=2, space=bass.MemorySpace.PSUM)
        )

        ipt = const.tile([Q, C, CI, NLOC], BF16)
        nc.sync.dma_start(ipt[:], ip_d.ap())
        iat = []
        for t in range(NT):
            it = const.tile([128, C, CI, Q], BF16, name=f"ia{t}")
            nc.sync.dma_start(it[:], ia_d.ap()[t])
            iat.append(it)
        spt = const.tile([Q, CI, NLOC], BF16)
        nc.sync.dma_start(spt[:], sp_d.ap())
        xst = const.tile([Q, B, C, CI], F32)
        nc.sync.dma_start(xst[:], xsc_d.ap())
        bmt = []
        for d in range(3):
            bm = const.tile([Q, Q], BF16, name=f"bm{d}")
            nc.sync.dma_start(bm[:], bm_d.ap()[d])
            bmt.append(bm)
        ones = const.tile([128, 32], BF16)
        nc.gpsimd.memset(ones[:], 1.0)

        # u tiles with zero guard slots 0 and CI+1 (zero padding for box3)
        ue = []
        for i in range(2):
            u = const.tile([Q, CI + 2, NLOC], BF16, name=f"ue{i}")
            nc.gpsimd.memset(u[:, 0], 0.0)
            nc.gpsimd.memset(u[:, CI + 1], 0.0)
            ue.append(u)

        for b in range(B):
            u = ue[b % 2]
            # t_c = I_c * x'_c  (P-major; per-partition scalar muls)
            tcs = work.tile([Q, C, CI, NLOC], BF16, tag="tcs")
            for c in range(C):
                for ci in range(CI):
                    eng = _MUL_ENG[c * CI + ci]
                    dst = tcs[:, c, ci]
                    src = ipt[:, c, ci]
                    sc = xst[:, b, c, ci : ci + 1]
                    if eng == "d":
                        nc.vector.tensor_scalar_mul(dst, src, sc)
                    elif eng == "a":
                        nc.scalar.mul(dst, src, sc)
                    else:
                        nc.gpsimd.tensor_scalar_mul(dst, src, sc)

            z01 = work.tile([Q, CI, NLOC], BF16, tag="z01")
            nc.vector.tensor_add(z01[:], tcs[:, 0], tcs[:, 1])
            z012 = work.tile([Q, CI, NLOC], BF16, tag="z012")
            nc.vector.tensor_add(z012[:], z01[:], tcs[:, 2])
            nc.vector.tensor_add(u[:, 1 : CI + 1], z012[:], spt[:])

            # box3 on PE: arg[ci] = sum_delta B_delta . u[ci+delta]; then exp.
            wp = wpool.tile([Q, CI, NLOC], BF16, tag="wp")
            for quarter in range(4):
                base = 1 + 2 * quarter  # u slot of first output chunk
                ap_ = psum.tile([Q, 2, NLOC], F32, tag="argp")
                for di in range(3):
                    delta = di - 1
                    s = base + delta
                    nc.tensor.matmul(
                        ap_[:].rearrange("q a n -> q (a n)"),
                        bmt[di][:],
                        u[:, s : s + 2].rearrange("q a n -> q (a n)"),
                        start=(di == 0),
                        stop=(di == 2),
                    )
                nc.scalar.activation(
                    wp[:, 2 * quarter : 2 * quarter + 2].rearrange(
                        "q a n -> q (a n)"
                    ),
                    ap_[:].rearrange("q a n -> q (a n)"),
                    AF.Exp,
                )

            # transpose w to A-major: wa[nl, ci*2+t, q] = wp[q, ci, t*128+nl]
            wa = wpool.tile([128, CI * NT, 128], BF16, tag="wa")
            nc.sync.dma_start_transpose(
                wa[:], wp[:].rearrange("q ci n -> q (ci n)")
            )
            wav = wa[:].rearrange("nl (ci t) q -> nl t ci q", t=NT)

            # v_c = w * I_c and ones-matmul reductions over n.
            # Quantity r = 2*qnt+half lands in PSUM region
            # rows [32*(r//3), +32), free [512*(r%3), +512) (redundant rows).
            rp = rpsum.tile([96, 3, 512], F32, tag="red")

            def red_out(r):
                pr = 32 * (r // 3)
                return rp[pr : pr + 32, r % 3].rearrange(
                    "p (a q) -> p a q", a=4
                )

            for t in range(NT):
                wv = wav[:, t]  # [128, CI, Q] strided
                v = work.tile([128, C, CI, Q], BF16, tag="v")
                for c in range(C):
                    nc.vector.tensor_mul(v[:, c], wv, iat[t][:, c])
                first, last = (t == 0), (t == NT - 1)
                for half in range(2):
                    cs_ = slice(4 * half, 4 * half + 4)
                    nc.tensor.matmul(
                        red_out(half), ones[:], wv[:, cs_],
                        start=first, stop=last,
                    )
                    for c in range(C):
                        nc.tensor.matmul(
                            red_out(2 + 2 * c + half), ones[:], v[:, c, cs_],
                            start=first, stop=last,
                        )
            # rows (0,32,64) x free-thirds -> out[b, 0:9, :] (row 8 unused)
            osb = work.tile([3, 3, 512], F32, tag="osb")
            nc.gpsimd.tensor_copy(
                osb[:], rp[:].rearrange("(g p) f q -> g p f q", p=32)[:, 0]
            )
            nc.sync.dma_start(
                out_d.ap()[b].rearrange("(g f) q -> g f q", f=3), osb[:]
            )

    nc.compile()
    return nc


def kernel(x, images, mu, sigma, t):
    x = np.ascontiguousarray(np.asarray(x, dtype=np.float32))
    images = np.ascontiguousarray(np.asarray(images, dtype=np.float32))
    m = float(np.asarray(mu)[int(t)])
    sig = float(np.asarray(sigma)[int(t)])
    sig2 = sig * sig
    c_s = -(m * m) / (2.0 * sig2)

    key = ()
    if key not in _cache:
        _cache[key] = _build()
    nc = _cache[key]

    xp = x.reshape(B, C, P) * (m / sig2)
    # xsc[q, b, c, ci] = x'[b, c, ci*128+q]
    xsc = np.ascontiguousarray(
        xp.reshape(B, C, CI, Q).transpose(3, 0, 1, 2), dtype=np.float32
    )
    bmats = _box_mats().astype(BF_NP)

    imgs = images.reshape(N, C, P)
    in_maps = []
    for k in range(NCORES):
        il = imgs[k * NLOC : (k + 1) * NLOC]  # [256, 3, 1024]
        ilb = il.astype(BF_NP)
        ip = np.ascontiguousarray(
            ilb.reshape(NLOC, C, CI, Q).transpose(3, 1, 2, 0)
        )  # [q, c, ci, n]
        ia = np.ascontiguousarray(ilb.reshape(NT, 128, C, CI, Q))
        s2 = c_s * (il.astype(np.float64) ** 2).sum(axis=1)  # [256, 1024]
        sp = np.ascontiguousarray(
            s2.reshape(NLOC, CI, Q).transpose(2, 1, 0).astype(BF_NP)
        )  # [q, ci, n]
        in_maps.append({"ip": ip, "ia": ia, "sp": sp, "xsc": xsc, "bm": bmats})

    import os

    trace = bool(os.environ.get("KERNEL_TRACE"))
    res = bass_utils.run_bass_kernel_spmd(
        nc, in_maps, core_ids=list(range(NCORES)), trace=trace
    )
    global _last_res
    _last_res = res
    parts = np.stack(
        [res.results[k]["out"] for k in range(NCORES)]
    )  # [8, B, 8, 512]
    tot = parts.astype(np.float64).sum(axis=0)  # [B, 8, 512]
    sw = tot[:, 0:2].reshape(B, P)
    swi = tot[:, 2:8].reshape(B, C, P)
    score = (m * swi / sw[:, None, :] - x.reshape(B, C, P)) / sig2
    return score.reshape(B, C, H, W).astype(np.float32)


# revision 17
# speedup vs baseline: 2.7489x; 1.0340x over previous
"""LocalScoreMachine Trainium2 kernel (pixel-major frontend + PE box filter).

score[b,c,p] = (m*SWI_c/SW - x[b,c,p]) / sig2, where over the dataset axis n:
    SW  = sum_n w,  SWI_c = sum_n w*I_c,
    w   = exp(box3(u)),  u = sum_c I_c * x'_c + c_s*S,
    x'  = x*(m/sig2),  S = sum_c I_c^2,  c_s = -m^2/(2*sig2).
(The b- and n-constant exp factors cancel in the SWI/SW ratio, and the
3x3 box filter is linear, so box3(u) gives the exact exponent up to that
constant.)

Sharding: dataset axis N=2048 -> 256 images per core (8 cores).

Per-core dataflow:
  *P-major phase* (partitions = 128 pixels of a 4-row chunk, free = n):
    - t_c = I_c * x'_c via tensor_scalar muls (per-partition scalar = x'),
      which hit the DVE 4x perf mode in bf16.  Split across DVE/Act/Pool.
    - u = t0+t1+t2+S'' via wide tensor adds (free = all 8 chunks * 256 n).
    - box3 via PE matmuls: block-tridiagonal 0/1 band matrices B_delta
      [128x128] couple chunk ci with chunks ci+delta; zero guard slots
      implement the zero padding.  Accumulated in PSUM.
    - w = exp(arg) on the Act engine, PSUM -> SBUF bf16.
  *Transpose*: one SBUF->SBUF dma_start_transpose per b flips w to
    image-major layout [n partitions, pixel free].
  *A-major phase* (partitions = n): v_c = w*I_c muls, then ones-matmul
    reductions over n on the PE into PSUM, accumulated over both n-tiles.
Host combines the 8 partial (SW, SWI) results and forms the score.
"""

import sys

for _p in ("/opt/trn_rl_repo", "/opt/trn_rl_repo/concourse", "/opt/pypackages"):
    if _p not in sys.path:
        sys.path.append(_p)

from contextlib import ExitStack

import numpy as np
import ml_dtypes

import concourse.bass as bass
import concourse.bacc as bacc
import concourse.mybir as mybir
import concourse.tile as tile
from concourse import bass_utils

B, N, C, H, W = 8, 2048, 3, 32, 32
P = H * W  # 1024 pixels
NCORES = 8
NLOC = N // NCORES  # 256 images per core
NT = 2  # n-tiles (128 partitions) per core on the A-major side
CI = 8  # pixel chunks (4 h-rows x 32 w = 128 pixels each)
Q = 128  # pixels per chunk
F32 = mybir.dt.float32
BF16 = mybir.dt.bfloat16
AF = mybir.ActivationFunctionType
BF_NP = ml_dtypes.bfloat16

_cache = {}
_last_res = None

# Engine assignment for the 24 per-b tensor_scalar muls (c, ci), tuned so
# DVE/Act/Pool finish together (DVE ~127ns/op, Act ~398, Pool ~451).
# index = c * CI + ci -> "d" (DVE) / "a" (Act) / "p" (Pool)
_MUL_ENG = list("dapdapdd" "apdapdap" "dapdapdd")


def _box_mats():
    """B_delta[k, m] = 1 if source pixel k (in chunk ci+delta) is in the
    3x3 neighborhood of target pixel m (in chunk ci); mats[3] = identity
    (used to seed the PSUM accumulation with the precomputed Bs term)."""
    q = np.arange(Q)
    hk, wk = q // W, q % W  # chunk-local h (0..3), w (0..31)
    hm, wm = hk, wk
    mats = []
    for delta in (-1, 0, 1):
        dh = 4 * delta + hk[:, None] - hm[None, :]
        dw = wk[:, None] - wm[None, :]
        mats.append(((np.abs(dh) <= 1) & (np.abs(dw) <= 1)).astype(np.float32))
    mats.append(np.eye(Q, dtype=np.float32))
    return np.stack(mats)  # [4, 128, 128] (k, m)


def _box3(a):
    """3x3 zero-padded box sum over the last two dims."""
    Hh, Ww = a.shape[-2], a.shape[-1]
    p = np.pad(a, [(0, 0)] * (a.ndim - 2) + [(1, 1), (1, 1)])
    return sum(
        p[..., i : i + Hh, j : j + Ww] for i in range(3) for j in range(3)
    )


def _build():
    nc = bacc.Bacc("TRN2", target_bir_lowering=False, debug=False)

    ip_d = nc.dram_tensor("ip", [Q, C, CI, NLOC], BF16, kind="ExternalInput")
    ia_d = nc.dram_tensor("ia", [NT, 128, C, CI, Q], BF16, kind="ExternalInput")
    bs_d = nc.dram_tensor("bs", [Q, CI, NLOC], BF16, kind="ExternalInput")
    xsc_d = nc.dram_tensor("xsc", [Q, B, C, CI], F32, kind="ExternalInput")
    bm_d = nc.dram_tensor("bm", [4, Q, Q], BF16, kind="ExternalInput")
    out_d = nc.dram_tensor("out", [B, 9, 512], F32, kind="ExternalOutput")

    with tile.TileContext(nc) as tc, ExitStack() as ctx:
        const = ctx.enter_context(tc.tile_pool(name="const", bufs=1))
        work = ctx.enter_context(tc.tile_pool(name="work", bufs=3))
        wpool = ctx.enter_context(tc.tile_pool(name="wpool", bufs=3))
        psum = ctx.enter_context(
            tc.tile_pool(name="psum", bufs=2, space=bass.MemorySpace.PSUM)
        )
        rpsum = ctx.enter_context(
            tc.tile_pool(name="rpsum", bufs=2, space=bass.MemorySpace.PSUM)
        )

        ipt = const.tile([Q, C, CI, NLOC], BF16)
        nc.sync.dma_start(ipt[:], ip_d.ap())
        iat = []
        for t in range(NT):
            it = const.tile([128, C, CI, Q], BF16, name=f"ia{t}")
            nc.sync.dma_start(it[:], ia_d.ap()[t])
            iat.append(it)
        bst = const.tile([Q, CI, NLOC], BF16)
        nc.sync.dma_start(bst[:], bs_d.ap())
        xst = const.tile([Q, B, C, CI], F32)
        nc.sync.dma_start(xst[:], xsc_d.ap())
        bmt = []
        for d in range(4):
            bm = const.tile([Q, Q], BF16, name=f"bm{d}")
            nc.sync.dma_start(bm[:], bm_d.ap()[d])
            bmt.append(bm)
        ones = const.tile([128, 32], BF16)
        nc.gpsimd.memset(ones[:], 1.0)

        # u tiles with zero guard slots 0 and CI+1 (zero padding for box3)
        ue = []
        for i in range(2):
            u = const.tile([Q, CI + 2, NLOC], BF16, name=f"ue{i}")
            nc.gpsimd.memset(u[:, 0], 0.0)
            nc.gpsimd.memset(u[:, CI + 1], 0.0)
            ue.append(u)

        for b in range(B):
            u = ue[b % 2]
            # t_c = I_c * x'_c  (P-major; per-partition scalar muls)
            tcs = work.tile([Q, C, CI, NLOC], BF16, tag="tcs")
            for c in range(C):
                for ci in range(CI):
                    eng = _MUL_ENG[c * CI + ci]
                    dst = tcs[:, c, ci]
                    src = ipt[:, c, ci]
                    sc = xst[:, b, c, ci : ci + 1]
                    if eng == "d":
                        nc.vector.tensor_scalar_mul(dst, src, sc)
                    elif eng == "a":
                        nc.scalar.mul(dst, src, sc)
                    else:
                        nc.gpsimd.tensor_scalar_mul(dst, src, sc)

            z01 = work.tile([Q, CI, NLOC], BF16, tag="z01")
            nc.vector.tensor_add(z01[:], tcs[:, 0], tcs[:, 1])
            nc.vector.tensor_add(u[:, 1 : CI + 1], z01[:], tcs[:, 2])

            # box3 on PE: arg[ci] = Bs[ci] + sum_delta B_delta . u[ci+delta]
            wp = wpool.tile([Q, CI, NLOC], BF16, tag="wp")
            for quarter in range(4):
                base = 1 + 2 * quarter  # u slot of first output chunk
                ap_ = psum.tile([Q, 2, NLOC], F32, tag="argp")
                nc.tensor.matmul(
                    ap_[:].rearrange("q a n -> q (a n)"),
                    bmt[3][:],
                    bst[:, 2 * quarter : 2 * quarter + 2].rearrange(
                        "q a n -> q (a n)"
                    ),
                    start=True,
                    stop=False,
                )
                for di in range(3):
                    delta = di - 1
                    s = base + delta
                    nc.tensor.matmul(
                        ap_[:].rearrange("q a n -> q (a n)"),
                        bmt[di][:],
                        u[:, s : s + 2].rearrange("q a n -> q (a n)"),
                        start=False,
                        stop=(di == 2),
                    )
                nc.scalar.activation(
                    wp[:, 2 * quarter : 2 * quarter + 2].rearrange(
                        "q a n -> q (a n)"
                    ),
                    ap_[:].rearrange("q a n -> q (a n)"),
                    AF.Exp,
                )

            # transpose w to A-major: wa[nl, ci*2+t, q] = wp[q, ci, t*128+nl]
            wa = wpool.tile([128, CI * NT, 128], BF16, tag="wa")
            nc.sync.dma_start_transpose(
                wa[:], wp[:].rearrange("q ci n -> q (ci n)")
            )
            wav = wa[:].rearrange("nl (ci t) q -> nl t ci q", t=NT)

            # v_c = w * I_c and ones-matmul reductions over n.
            # Quantity r = 2*qnt+half lands in PSUM region
            # rows [32*(r//3), +32), free [512*(r%3), +512) (redundant rows).
            rp = rpsum.tile([96, 3, 512], F32, tag="red")

            def red_out(r):
                pr = 32 * (r // 3)
                return rp[pr : pr + 32, r % 3].rearrange(
                    "p (a q) -> p a q", a=4
                )

            for t in range(NT):
                wv = wav[:, t]  # [128, CI, Q] strided
                wvb = wv.rearrange("nl (a one) q -> nl one a q", one=1).broadcast_to(
                    [128, C, CI, Q]
                )
                v = work.tile([128, C, CI, Q], BF16, tag="v")
                nc.vector.tensor_mul(v[:], wvb, iat[t][:])
                first, last = (t == 0), (t == NT - 1)
                for half in range(2):
                    cs_ = slice(4 * half, 4 * half + 4)
                    nc.tensor.matmul(
                        red_out(half), ones[:], wv[:, cs_],
                        start=first, stop=last,
                    )
                    for c in range(C):
                        nc.tensor.matmul(
                            red_out(2 + 2 * c + half), ones[:], v[:, c, cs_],
                            start=first, stop=last,
                        )
            # rows (0,32,64) x free-thirds -> out[b, 0:9, :] (row 8 unused)
            osb = work.tile([3, 3, 512], F32, tag="osb")
            rp0 = rp[:].rearrange("(g p) f q -> g p f q", p=32)[:, 0]
            nc.scalar.copy(
                osb[:, 0:2].rearrange("g f q -> g (f q)"),
                rp0[:, 0:2].rearrange("g f q -> g (f q)"),
            )
            nc.gpsimd.tensor_copy(osb[:, 2], rp0[:, 2])
            nc.sync.dma_start(
                out_d.ap()[b].rearrange("(g f) q -> g f q", f=3), osb[:]
            )

    nc.compile()
    return nc


def kernel(x, images, mu, sigma, t):
    x = np.ascontiguousarray(np.asarray(x, dtype=np.float32))
    images = np.ascontiguousarray(np.asarray(images, dtype=np.float32))
    m = float(np.asarray(mu)[int(t)])
    sig = float(np.asarray(sigma)[int(t)])
    sig2 = sig * sig
    c_s = -(m * m) / (2.0 * sig2)

    key = ()
    if key not in _cache:
        _cache[key] = _build()
    nc = _cache[key]

    xp = x.reshape(B, C, P) * (m / sig2)
    # xsc[q, b, c, ci] = x'[b, c, ci*128+q]
    xsc = np.ascontiguousarray(
        xp.reshape(B, C, CI, Q).transpose(3, 0, 1, 2), dtype=np.float32
    )
    bmats = _box_mats().astype(BF_NP)

    imgs = images.reshape(N, C, P)
    in_maps = []
    for k in range(NCORES):
        il = imgs[k * NLOC : (k + 1) * NLOC]  # [256, 3, 1024]
        ilb = il.astype(BF_NP)
        ip = np.ascontiguousarray(
            ilb.reshape(NLOC, C, CI, Q).transpose(3, 1, 2, 0)
        )  # [q, c, ci, n]
        ia = np.ascontiguousarray(ilb.reshape(NT, 128, C, CI, Q))
        s2 = c_s * (il.astype(np.float64) ** 2).sum(axis=1)  # [256, 1024]
        bsv = _box3(s2.reshape(NLOC, H, W)).reshape(NLOC, P)
        bs = np.ascontiguousarray(
            bsv.reshape(NLOC, CI, Q).transpose(2, 1, 0).astype(BF_NP)
        )  # [q, ci, n]
        in_maps.append({"ip": ip, "ia": ia, "bs": bs, "xsc": xsc, "bm": bmats})

    import os

    trace = bool(os.environ.get("KERNEL_TRACE"))
    res = bass_utils.run_bass_kernel_spmd(
        nc, in_maps, core_ids=list(range(NCORES)), trace=trace
    )
    global _last_res
    _last_res = res
    parts = np.stack(
        [res.results[k]["out"] for k in range(NCORES)]
    )  # [8, B, 8, 512]
    tot = parts.astype(np.float64).sum(axis=0)  # [B, 8, 512]
    sw = tot[:, 0:2].reshape(B, P)
    swi = tot[:, 2:8].reshape(B, C, P)
    score = (m * swi / sw[:, None, :] - x.reshape(B, C, P)) / sig2
    return score.reshape(B, C, H, W).astype(np.float32)


# revision 21
# speedup vs baseline: 3.3216x; 1.2084x over previous
"""LocalScoreMachine Trainium2 kernel (pixel-major frontend + PE box filter).

score[b,c,p] = (m*SWI_c/SW - x[b,c,p]) / sig2, where over the dataset axis n:
    SW  = sum_n w,  SWI_c = sum_n w*I_c,
    w   = exp(box3(u)),  u = sum_c I_c * x'_c + c_s*S,
    x'  = x*(m/sig2),  S = sum_c I_c^2,  c_s = -m^2/(2*sig2).
(The b- and n-constant exp factors cancel in the SWI/SW ratio, and the
3x3 box filter is linear, so box3(u) gives the exact exponent up to that
constant.)

Sharding: dataset axis N=2048 -> 256 images per core (8 cores).

Per-core dataflow:
  *P-major phase* (partitions = 128 pixels of a 4-row chunk, free = n):
    - t_c = I_c * x'_c via tensor_scalar muls (per-partition scalar = x'),
      which hit the DVE 4x perf mode in bf16.  Split across DVE/Act/Pool.
    - u = t0+t1+t2+S'' via wide tensor adds (free = all 8 chunks * 256 n).
    - box3 via PE matmuls: block-tridiagonal 0/1 band matrices B_delta
      [128x128] couple chunk ci with chunks ci+delta; zero guard slots
      implement the zero padding.  Accumulated in PSUM.
    - w = exp(arg) on the Act engine, PSUM -> SBUF bf16.
  *Transpose*: one SBUF->SBUF dma_start_transpose per b flips w to
    image-major layout [n partitions, pixel free].
  *A-major phase* (partitions = n): v_c = w*I_c muls, then ones-matmul
    reductions over n on the PE into PSUM, accumulated over both n-tiles.
Host combines the 8 partial (SW, SWI) results and forms the score.
"""

import sys

for _p in ("/opt/trn_rl_repo", "/opt/trn_rl_repo/concourse", "/opt/pypackages"):
    if _p not in sys.path:
        sys.path.append(_p)

from contextlib import ExitStack

import numpy as np
import ml_dtypes

import concourse.bass as bass
import concourse.bacc as bacc
import concourse.mybir as mybir
import concourse.tile as tile
from concourse import bass_utils

B, N, C, H, W = 8, 2048, 3, 32, 32
P = H * W  # 1024 pixels
NCORES = 8
NLOC = N // NCORES  # 256 images per core
NT = 2  # n-tiles (128 partitions) per core on the A-major side
CI = 8  # pixel chunks (4 h-rows x 32 w = 128 pixels each)
Q = 128  # pixels per chunk
F32 = mybir.dt.float32
BF16 = mybir.dt.bfloat16
AF = mybir.ActivationFunctionType
BF_NP = ml_dtypes.bfloat16

_cache = {}
_last_res = None

# Engine assignment for the 24 per-b tensor_scalar muls (c, ci), tuned so
# DVE/Act/Pool finish together (DVE ~127ns/op, Act ~398, Pool ~451).
# index = c * CI + ci -> "d" (DVE) / "a" (Act) / "p" (Pool)
_MUL_ENG = list("dapdapdd" "apdapdap" "dapdapdd")


def _box_mats():
    """B_delta[k, m] = 1 if source pixel k (in chunk ci+delta) is in the
    3x3 neighborhood of target pixel m (in chunk ci); mats[3] = identity
    (used to seed the PSUM accumulation with the precomputed Bs term)."""
    q = np.arange(Q)
    hk, wk = q // W, q % W  # chunk-local h (0..3), w (0..31)
    hm, wm = hk, wk
    mats = []
    for delta in (-1, 0, 1):
        dh = 4 * delta + hk[:, None] - hm[None, :]
        dw = wk[:, None] - wm[None, :]
        mats.append(((np.abs(dh) <= 1) & (np.abs(dw) <= 1)).astype(np.float32))
    mats.append(np.eye(Q, dtype=np.float32))
    return np.stack(mats)  # [4, 128, 128] (k, m)


def _box3(a):
    """3x3 zero-padded box sum over the last two dims."""
    Hh, Ww = a.shape[-2], a.shape[-1]
    p = np.pad(a, [(0, 0)] * (a.ndim - 2) + [(1, 1), (1, 1)])
    return sum(
        p[..., i : i + Hh, j : j + Ww] for i in range(3) for j in range(3)
    )


def _build():
    nc = bacc.Bacc("TRN2", target_bir_lowering=False, debug=False)

    ip_d = nc.dram_tensor("ip", [Q, C, CI, NLOC], BF16, kind="ExternalInput")
    ia_d = nc.dram_tensor("ia", [NT, 128, C, CI, Q], BF16, kind="ExternalInput")
    bs_d = nc.dram_tensor("bs", [Q, CI, NLOC], BF16, kind="ExternalInput")
    xsc_d = nc.dram_tensor("xsc", [Q, B, C, CI], F32, kind="ExternalInput")
    bm_d = nc.dram_tensor("bm", [4, Q, Q], BF16, kind="ExternalInput")
    out_d = nc.dram_tensor("out", [B, 9, 512], F32, kind="ExternalOutput")

    with tile.TileContext(nc) as tc, ExitStack() as ctx:
        const = ctx.enter_context(tc.tile_pool(name="const", bufs=1))
        work = ctx.enter_context(tc.tile_pool(name="work", bufs=3))
        wpool = ctx.enter_context(tc.tile_pool(name="wpool", bufs=3))
        psum = ctx.enter_context(
            tc.tile_pool(name="psum", bufs=2, space=bass.MemorySpace.PSUM)
        )
        rpsum = ctx.enter_context(
            tc.tile_pool(name="rpsum", bufs=2, space=bass.MemorySpace.PSUM)
        )

        # Load order matters: the b=0 muls need xst + ipt first.
        xst = const.tile([Q, B, C, CI], F32)
        nc.sync.dma_start(xst[:], xsc_d.ap())
        ipt = const.tile([Q, C, CI, NLOC], BF16)
        for c in range(C):
            nc.sync.dma_start(ipt[:, c], ip_d.ap()[:, c])
        bmt = []
        for d in range(4):
            bm = const.tile([Q, Q], BF16, name=f"bm{d}")
            nc.sync.dma_start(bm[:], bm_d.ap()[d])
            bmt.append(bm)
        bst = const.tile([Q, CI, NLOC], BF16)
        nc.sync.dma_start(bst[:], bs_d.ap())
        iat = []
        for t in range(NT):
            it = const.tile([128, C, CI, Q], BF16, name=f"ia{t}")
            nc.sync.dma_start(it[:], ia_d.ap()[t])
            iat.append(it)
        ones = const.tile([128, 32], BF16)
        nc.gpsimd.memset(ones[:], 1.0)

        # u tiles with zero guard slots 0 and CI+1 (zero padding for box3)
        ue = []
        for i in range(2):
            u = const.tile([Q, CI + 2, NLOC], BF16, name=f"ue{i}")
            nc.gpsimd.memset(u[:, 0], 0.0)
            nc.gpsimd.memset(u[:, CI + 1], 0.0)
            ue.append(u)

        was = {}

        def front(b):
            u = ue[b % 2]
            # t_c = I_c * x'_c  (P-major; per-partition scalar muls)
            tcs = work.tile([Q, C, CI, NLOC], BF16, tag="tcs")
            for c in range(C):
                for ci in range(CI):
                    eng = _MUL_ENG[c * CI + ci]
                    dst = tcs[:, c, ci]
                    src = ipt[:, c, ci]
                    sc = xst[:, b, c, ci : ci + 1]
                    if eng == "d":
                        nc.vector.tensor_scalar_mul(dst, src, sc)
                    elif eng == "a":
                        nc.scalar.mul(dst, src, sc)
                    else:
                        nc.gpsimd.tensor_scalar_mul(dst, src, sc)

            z01 = work.tile([Q, CI, NLOC], BF16, tag="z01")
            nc.vector.tensor_add(z01[:], tcs[:, 0], tcs[:, 1])
            nc.vector.tensor_add(u[:, 1 : CI + 1], z01[:], tcs[:, 2])

            # box3 on PE: arg[ci] = Bs[ci] + sum_delta B_delta . u[ci+delta]
            wp = wpool.tile([Q, CI, NLOC], BF16, tag="wp")
            for quarter in range(4):
                base = 1 + 2 * quarter  # u slot of first output chunk
                ap_ = psum.tile([Q, 2, NLOC], F32, tag="argp")
                nc.tensor.matmul(
                    ap_[:].rearrange("q a n -> q (a n)"),
                    bmt[3][:],
                    bst[:, 2 * quarter : 2 * quarter + 2].rearrange(
                        "q a n -> q (a n)"
                    ),
                    start=True,
                    stop=False,
                )
                for di in range(3):
                    delta = di - 1
                    s = base + delta
                    nc.tensor.matmul(
                        ap_[:].rearrange("q a n -> q (a n)"),
                        bmt[di][:],
                        u[:, s : s + 2].rearrange("q a n -> q (a n)"),
                        start=False,
                        stop=(di == 2),
                    )
                nc.scalar.activation(
                    wp[:, 2 * quarter : 2 * quarter + 2].rearrange(
                        "q a n -> q (a n)"
                    ),
                    ap_[:].rearrange("q a n -> q (a n)"),
                    AF.Exp,
                )

            # transpose w to A-major: wa[nl, ci*2+t, q] = wp[q, ci, t*128+nl]
            wa = wpool.tile([128, CI * NT, 128], BF16, tag="wa")
            nc.sync.dma_start_transpose(
                wa[:], wp[:].rearrange("q ci n -> q (ci n)")
            )
            was[b] = wa

        def back(b):
            wav = was.pop(b)[:].rearrange("nl (ci t) q -> nl t ci q", t=NT)

            # v_c = w * I_c and ones-matmul reductions over n.
            # Quantity r = 2*qnt+half lands in PSUM region
            # rows [32*(r//3), +32), free [512*(r%3), +512) (redundant rows).
            rp = rpsum.tile([96, 3, 512], F32, tag="red")

            def red_out(r):
                pr = 32 * (r // 3)
                return rp[pr : pr + 32, r % 3].rearrange(
                    "p (a q) -> p a q", a=4
                )

            for t in range(NT):
                wv = wav[:, t]  # [128, CI, Q] strided
                wvb = wv.rearrange("nl (a one) q -> nl one a q", one=1).broadcast_to(
                    [128, C, CI, Q]
                )
                v = work.tile([128, C, CI, Q], BF16, tag="v")
                nc.vector.tensor_mul(v[:], wvb, iat[t][:])
                first, last = (t == 0), (t == NT - 1)
                for half in range(2):
                    cs_ = slice(4 * half, 4 * half + 4)
                    nc.tensor.matmul(
                        red_out(half), ones[:], wv[:, cs_],
                        start=first, stop=last,
                    )
                    for c in range(C):
                        nc.tensor.matmul(
                            red_out(2 + 2 * c + half), ones[:], v[:, c, cs_],
                            start=first, stop=last,
                        )
            # rows (0,32,64) x free-thirds -> out[b, 0:9, :] (row 8 unused)
            osb = work.tile([3, 3, 512], F32, tag="osb")
            rp0 = rp[:].rearrange("(g p) f q -> g p f q", p=32)[:, 0]
            nc.scalar.copy(
                osb[:, 0:2].rearrange("g f q -> g (f q)"),
                rp0[:, 0:2].rearrange("g f q -> g (f q)"),
            )
            nc.gpsimd.tensor_copy(osb[:, 2], rp0[:, 2])
            nc.sync.dma_start(
                out_d.ap()[b].rearrange("(g f) q -> g f q", f=3), osb[:]
            )

        # Software pipeline: the A-major back half of iteration b runs one
        # step behind its front half so the w-transpose DMA round trip is
        # hidden behind the next iteration's front-half work.
        for step in range(B + 1):
            if step < B:
                front(step)
            if step >= 1:
                back(step - 1)

    nc.compile()
    return nc


def kernel(x, images, mu, sigma, t):
    x = np.ascontiguousarray(np.asarray(x, dtype=np.float32))
    images = np.ascontiguousarray(np.asarray(images, dtype=np.float32))
    m = float(np.asarray(mu)[int(t)])
    sig = float(np.asarray(sigma)[int(t)])
    sig2 = sig * sig
    c_s = -(m * m) / (2.0 * sig2)

    key = ()
    if key not in _cache:
        _cache[key] = _build()
    nc = _cache[key]

    xp = x.reshape(B, C, P) * (m / sig2)
    # xsc[q, b, c, ci] = x'[b, c, ci*128+q]
    xsc = np.ascontiguousarray(
        xp.reshape(B, C, CI, Q).transpose(3, 0, 1, 2), dtype=np.float32
    )
    bmats = _box_mats().astype(BF_NP)

    imgs = images.reshape(N, C, P)
    in_maps = []
    for k in range(NCORES):
        il = imgs[k * NLOC : (k + 1) * NLOC]  # [256, 3, 1024]
        ilb = il.astype(BF_NP)
        ip = np.ascontiguousarray(
            ilb.reshape(NLOC, C, CI, Q).transpose(3, 1, 2, 0)
        )  # [q, c, ci, n]
        ia = np.ascontiguousarray(ilb.reshape(NT, 128, C, CI, Q))
        s2 = c_s * (il.astype(np.float64) ** 2).sum(axis=1)  # [256, 1024]
        bsv = _box3(s2.reshape(NLOC, H, W)).reshape(NLOC, P)
        bs = np.ascontiguousarray(
            bsv.reshape(NLOC, CI, Q).transpose(2, 1, 0).astype(BF_NP)
        )  # [q, ci, n]
        in_maps.append({"ip": ip, "ia": ia, "bs": bs, "xsc": xsc, "bm": bmats})

    import os

    trace = bool(os.environ.get("KERNEL_TRACE"))
    res = bass_utils.run_bass_kernel_spmd(
        nc, in_maps, core_ids=list(range(NCORES)), trace=trace
    )
    global _last_res
    _last_res = res
    parts = np.stack(
        [res.results[k]["out"] for k in range(NCORES)]
    )  # [8, B, 8, 512]
    tot = parts.astype(np.float64).sum(axis=0)  # [B, 8, 512]
    sw = tot[:, 0:2].reshape(B, P)
    swi = tot[:, 2:8].reshape(B, C, P)
    score = (m * swi / sw[:, None, :] - x.reshape(B, C, P)) / sig2
    return score.reshape(B, C, H, W).astype(np.float32)


# revision 25
# speedup vs baseline: 3.4893x; 1.0505x over previous
"""LocalScoreMachine Trainium2 kernel (pixel-major frontend + PE box filter).

score[b,c,p] = (m*SWI_c/SW - x[b,c,p]) / sig2, where over the dataset axis n:
    SW  = sum_n w,  SWI_c = sum_n w*I_c,
    w   = exp(box3(u)),  u = sum_c I_c * x'_c + c_s*S,
    x'  = x*(m/sig2),  S = sum_c I_c^2,  c_s = -m^2/(2*sig2).
(The b- and n-constant exp factors cancel in the SWI/SW ratio, and the
3x3 box filter is linear, so box3(u) gives the exact exponent up to that
constant.)

Sharding: dataset axis N=2048 -> 256 images per core (8 cores).

Per-core dataflow:
  *P-major phase* (partitions = 128 pixels of a 4-row chunk, free = n):
    - t_c = I_c * x'_c via tensor_scalar muls (per-partition scalar = x'),
      which hit the DVE 4x perf mode in bf16.  Split across DVE/Act/Pool.
    - u = t0+t1+t2+S'' via wide tensor adds (free = all 8 chunks * 256 n).
    - box3 via PE matmuls: block-tridiagonal 0/1 band matrices B_delta
      [128x128] couple chunk ci with chunks ci+delta; zero guard slots
      implement the zero padding.  Accumulated in PSUM.
    - w = exp(arg) on the Act engine, PSUM -> SBUF bf16.
  *Transpose*: one SBUF->SBUF dma_start_transpose per b flips w to
    image-major layout [n partitions, pixel free].
  *A-major phase* (partitions = n): v_c = w*I_c muls, then ones-matmul
    reductions over n on the PE into PSUM, accumulated over both n-tiles.
Host combines the 8 partial (SW, SWI) results and forms the score.
"""

import sys

for _p in ("/opt/trn_rl_repo", "/opt/trn_rl_repo/concourse", "/opt/pypackages"):
    if _p not in sys.path:
        sys.path.append(_p)

from contextlib import ExitStack

import numpy as np
import ml_dtypes

import concourse.bass as bass
import concourse.bacc as bacc
import concourse.mybir as mybir
import concourse.tile as tile
from concourse import bass_utils

B, N, C, H, W = 8, 2048, 3, 32, 32
P = H * W  # 1024 pixels
NCORES = 8
NLOC = N // NCORES  # 256 images per core
NT = 2  # n-tiles (128 partitions) per core on the A-major side
CI = 8  # pixel chunks (4 h-rows x 32 w = 128 pixels each)
Q = 128  # pixels per chunk
F32 = mybir.dt.float32
BF16 = mybir.dt.bfloat16
AF = mybir.ActivationFunctionType
BF_NP = ml_dtypes.bfloat16

_cache = {}
_last_res = None

# Engine assignment for the 24 per-b tensor_scalar muls (c, ci), tuned so
# DVE/Act/Pool finish together (DVE ~127ns/op, Act ~398, Pool ~451).
# index = c * CI + ci -> "d" (DVE) / "a" (Act) / "p" (Pool)
_MUL_ENG = list("dpadpdpd" "apdpddpa" "dpadpdpd")


def _box_mats():
    """B_delta[k, m] = 1 if source pixel k (in chunk ci+delta) is in the
    3x3 neighborhood of target pixel m (in chunk ci); mats[3] = identity
    (used to seed the PSUM accumulation with the precomputed Bs term)."""
    q = np.arange(Q)
    hk, wk = q // W, q % W  # chunk-local h (0..3), w (0..31)
    hm, wm = hk, wk
    mats = []
    for delta in (-1, 0, 1):
        dh = 4 * delta + hk[:, None] - hm[None, :]
        dw = wk[:, None] - wm[None, :]
        mats.append(((np.abs(dh) <= 1) & (np.abs(dw) <= 1)).astype(np.float32))
    mats.append(np.eye(Q, dtype=np.float32))
    return np.stack(mats)  # [4, 128, 128] (k, m)


def _box3(a):
    """3x3 zero-padded box sum over the last two dims."""
    Hh, Ww = a.shape[-2], a.shape[-1]
    p = np.pad(a, [(0, 0)] * (a.ndim - 2) + [(1, 1), (1, 1)])
    return sum(
        p[..., i : i + Hh, j : j + Ww] for i in range(3) for j in range(3)
    )


def _build():
    nc = bacc.Bacc("TRN2", target_bir_lowering=False, debug=False)

    ip_d = nc.dram_tensor("ip", [Q, C, CI, NLOC], BF16, kind="ExternalInput")
    ia_d = nc.dram_tensor("ia", [NT, 128, C, CI, Q], BF16, kind="ExternalInput")
    bs_d = nc.dram_tensor("bs", [Q, CI, NLOC], BF16, kind="ExternalInput")
    xsc_d = nc.dram_tensor("xsc", [Q, B, C, CI], F32, kind="ExternalInput")
    bm_d = nc.dram_tensor("bm", [4, Q, Q], BF16, kind="ExternalInput")
    out_d = nc.dram_tensor("out", [B, 9, 512], F32, kind="ExternalOutput")

    with tile.TileContext(nc) as tc, ExitStack() as ctx:
        const = ctx.enter_context(tc.tile_pool(name="const", bufs=1))
        work = ctx.enter_context(tc.tile_pool(name="work", bufs=3))
        wpool = ctx.enter_context(tc.tile_pool(name="wpool", bufs=3))
        psum = ctx.enter_context(
            tc.tile_pool(name="psum", bufs=2, space=bass.MemorySpace.PSUM)
        )
        rpsum = ctx.enter_context(
            tc.tile_pool(name="rpsum", bufs=2, space=bass.MemorySpace.PSUM)
        )

        # Load order matters: the b=0 muls need xst + ipt first.
        xst = const.tile([Q, B, C, CI], F32)
        nc.sync.dma_start(xst[:], xsc_d.ap())
        ipt = const.tile([Q, C, CI, NLOC], BF16)
        nc.sync.dma_start(ipt[:, 0], ip_d.ap()[:, 0])
        bmt = []
        for d in range(4):
            bm = const.tile([Q, Q], BF16, name=f"bm{d}")
            nc.sync.dma_start(bm[:], bm_d.ap()[d])
            bmt.append(bm)
        for c in range(1, C):
            nc.sync.dma_start(ipt[:, c], ip_d.ap()[:, c])
        bst = const.tile([Q, CI, NLOC], BF16)
        nc.sync.dma_start(bst[:], bs_d.ap())
        iat = []
        for t in range(NT):
            it = const.tile([128, C, CI, Q], BF16, name=f"ia{t}")
            nc.sync.dma_start(it[:], ia_d.ap()[t])
            iat.append(it)
        ones = const.tile([128, 32], BF16)
        nc.gpsimd.memset(ones[:], 1.0)

        # u tiles with zero guard slots 0 and CI+1 (zero padding for box3)
        ue = []
        for i in range(2):
            u = const.tile([Q, CI + 2, NLOC], BF16, name=f"ue{i}")
            nc.gpsimd.memset(u[:, 0], 0.0)
            nc.gpsimd.memset(u[:, CI + 1], 0.0)
            ue.append(u)

        was = {}

        def front(b):
            u = ue[b % 2]
            # t_c = I_c * x'_c  (P-major; per-partition scalar muls)
            tcs = work.tile([Q, C, CI, NLOC], BF16, tag="tcs")
            for c in range(C):
                for ci in range(CI):
                    eng = _MUL_ENG[c * CI + ci]
                    dst = tcs[:, c, ci]
                    src = ipt[:, c, ci]
                    sc = xst[:, b, c, ci : ci + 1]
                    if eng == "d":
                        nc.vector.tensor_scalar_mul(dst, src, sc)
                    elif eng == "a":
                        nc.scalar.mul(dst, src, sc)
                    else:
                        nc.gpsimd.tensor_scalar_mul(dst, src, sc)

            z01 = work.tile([Q, CI, NLOC], BF16, tag="z01")
            nc.vector.tensor_add(z01[:], tcs[:, 0], tcs[:, 1])
            nc.vector.tensor_add(u[:, 1 : CI + 1], z01[:], tcs[:, 2])

            # box3 on PE: arg[ci] = Bs[ci] + sum_delta B_delta . u[ci+delta]
            wp = wpool.tile([Q, CI, NLOC], BF16, tag="wp")
            wa = wpool.tile([128, CI * NT, 128], BF16, tag="wa")
            for quarter in range(4):
                base = 1 + 2 * quarter  # u slot of first output chunk
                ap_ = psum.tile([Q, 2, NLOC], F32, tag="argp")
                nc.tensor.matmul(
                    ap_[:].rearrange("q a n -> q (a n)"),
                    bmt[3][:],
                    bst[:, 2 * quarter : 2 * quarter + 2].rearrange(
                        "q a n -> q (a n)"
                    ),
                    start=True,
                    stop=False,
                )
                for di in range(3):
                    delta = di - 1
                    s = base + delta
                    nc.tensor.matmul(
                        ap_[:].rearrange("q a n -> q (a n)"),
                        bmt[di][:],
                        u[:, s : s + 2].rearrange("q a n -> q (a n)"),
                        start=False,
                        stop=(di == 2),
                    )
                nc.scalar.activation(
                    wp[:, 2 * quarter : 2 * quarter + 2].rearrange(
                        "q a n -> q (a n)"
                    ),
                    ap_[:].rearrange("q a n -> q (a n)"),
                    AF.Exp,
                )
                if quarter % 2 == 1:
                    # transpose this pixel-half of w to A-major:
                    # wa[nl, ci*2+t, q] = wp[q, ci, t*128+nl]
                    h = quarter // 2
                    nc.sync.dma_start_transpose(
                        wa[:, 8 * h : 8 * h + 8],
                        wp[:, 4 * h : 4 * h + 4].rearrange(
                            "q ci n -> q (ci n)"
                        ),
                    )
            was[b] = wa

        def back(b):
            wav = was.pop(b)[:].rearrange("nl (ci t) q -> nl t ci q", t=NT)

            # v_c = w * I_c and ones-matmul reductions over n.
            # Quantity r = 2*qnt+half lands in PSUM region
            # rows [32*(r//3), +32), free [512*(r%3), +512) (redundant rows).
            rp = rpsum.tile([96, 3, 512], F32, tag="red")

            def red_out(r):
                pr = 32 * (r // 3)
                return rp[pr : pr + 32, r % 3].rearrange(
                    "p (a q) -> p a q", a=4
                )

            for t in range(NT):
                first, last = (t == 0), (t == NT - 1)
                for half in range(2):
                    cs_ = slice(4 * half, 4 * half + 4)
                    wv = wav[:, t, cs_]  # [128, 4, Q] strided
                    wvb = wv.rearrange(
                        "nl (a one) q -> nl one a q", one=1
                    ).broadcast_to([128, C, 4, Q])
                    v = work.tile([128, C, 4, Q], BF16, tag="v")
                    nc.vector.tensor_mul(v[:], wvb, iat[t][:, :, cs_])
                    nc.tensor.matmul(
                        red_out(half), ones[:], wv,
                        start=first, stop=last,
                    )
                    for c in range(C):
                        nc.tensor.matmul(
                            red_out(2 + 2 * c + half), ones[:], v[:, c],
                            start=first, stop=last,
                        )
            # rows (0,32,64) x free-thirds -> out[b, 0:9, :] (row 8 unused)
            osb = work.tile([3, 3, 512], F32, tag="osb")
            rp0 = rp[:].rearrange("(g p) f q -> g p f q", p=32)[:, 0]
            nc.scalar.copy(
                osb[:, 0:2].rearrange("g f q -> g (f q)"),
                rp0[:, 0:2].rearrange("g f q -> g (f q)"),
            )
            nc.gpsimd.tensor_copy(osb[:, 2], rp0[:, 2])
            nc.sync.dma_start(
                out_d.ap()[b].rearrange("(g f) q -> g f q", f=3), osb[:]
            )

        # Software pipeline: the A-major back half of iteration b runs one
        # step behind its front half so the w-transpose DMA round trip is
        # hidden behind the next iteration's front-half work.
        for step in range(B + 1):
            if step < B:
                front(step)
            if step >= 1:
                back(step - 1)

    nc.compile()
    return nc


def kernel(x, images, mu, sigma, t):
    x = np.ascontiguousarray(np.asarray(x, dtype=np.float32))
    images = np.ascontiguousarray(np.asarray(images, dtype=np.float32))
    m = float(np.asarray(mu)[int(t)])
    sig = float(np.asarray(sigma)[int(t)])
    sig2 = sig * sig
    c_s = -(m * m) / (2.0 * sig2)

    key = ()
    if key not in _cache:
        _cache[key] = _build()
    nc = _cache[key]

    xp = x.reshape(B, C, P) * (m / sig2)
    # xsc[q, b, c, ci] = x'[b, c, ci*128+q]
    xsc = np.ascontiguousarray(
        xp.reshape(B, C, CI, Q).transpose(3, 0, 1, 2), dtype=np.float32
    )
    bmats = _box_mats().astype(BF_NP)

    imgs = images.reshape(N, C, P)
    in_maps = []
    for k in range(NCORES):
        il = imgs[k * NLOC : (k + 1) * NLOC]  # [256, 3, 1024]
        ilb = il.astype(BF_NP)
        ip = np.ascontiguousarray(
            ilb.reshape(NLOC, C, CI, Q).transpose(3, 1, 2, 0)
        )  # [q, c, ci, n]
        ia = np.ascontiguousarray(ilb.reshape(NT, 128, C, CI, Q))
        s2 = c_s * (il.astype(np.float64) ** 2).sum(axis=1)  # [256, 1024]
        bsv = _box3(s2.reshape(NLOC, H, W)).reshape(NLOC, P)
        bs = np.ascontiguousarray(
            bsv.reshape(NLOC, CI, Q).transpose(2, 1, 0).astype(BF_NP)
        )  # [q, ci, n]
        in_maps.append({"ip": ip, "ia": ia, "bs": bs, "xsc": xsc, "bm": bmats})

    import os

    trace = bool(os.environ.get("KERNEL_TRACE"))
    res = bass_utils.run_bass_kernel_spmd(
        nc, in_maps, core_ids=list(range(NCORES)), trace=trace
    )
    global _last_res
    _last_res = res
    parts = np.stack(
        [res.results[k]["out"] for k in range(NCORES)]
    )  # [8, B, 8, 512]
    tot = parts.astype(np.float64).sum(axis=0)  # [B, 8, 512]
    sw = tot[:, 0:2].reshape(B, P)
    swi = tot[:, 2:8].reshape(B, C, P)
    score = (m * swi / sw[:, None, :] - x.reshape(B, C, P)) / sig2
    return score.reshape(B, C, H, W).astype(np.float32)
